# revision 6
# baseline (speedup 1.0000x reference)
"""NTM cell (scatter_memory) Trainium2 Bass kernel.

Full-input contract: kernel(**inputs) takes the unsharded numpy inputs
(B=256, N=1024, M=H=256), shards the batch dim across 8 NeuronCores
(pure data parallelism, 32 batches/core), runs one SPMD Bass/Tile NEFF,
and gathers the full outputs (y, new_memory, new_read_w, new_write_w).

Per-core dataflow (natural [n, m] layout, fp32 throughout):
  A:   stream memory[b] slabs [128, 8, 256]; read_vec via per-b PE matvec
       (PSUM [1,256] accumulation), row norms via ACT Square+accum.
  GRU: 2 steps + projection entirely on-chip in [feat, batch] layout
       (PE matmuls + ACT gates), then transpose co back to [batch, feat].
  B1:  stream memory again; cosine dots via DVE tensor_tensor_reduce
       against PE-broadcast keys.
  ADDR: both heads' softmax/interp/shift/pow addressing on [32, 1024].
  B2:  stream memory a third time; new_memory = mem + wt*(add - mem*erase)
       via GpSimd broadcast-mult + DVE subtract + in-place fused
       scalar_tensor_tensor; store.
"""
import os
import sys
import functools

sys.path.insert(0, "/opt/trn_rl_repo")
sys.path.insert(0, "/opt/trn_rl_repo/concourse")

import numpy as np
from contextlib import ExitStack

import concourse.bass as bass
import concourse.bacc as bacc
import concourse.tile as tile
import concourse.masks as masks
from concourse import mybir

AL = mybir.AluOpType
AF = mybir.ActivationFunctionType
F32 = mybir.dt.float32
AX = mybir.AxisListType

EPS = 1e-12
B, N, M, H = 256, 1024, 256, 256
NCORES = 8
BL = B // NCORES          # 32 batches per core
NC = N // 128             # 8 n-chunks per batch
OUTF = 5 * M + 12         # 1292
RL = M + 6                # 262 read head params
# co column map: rh=[0:262], wh=[262:1036], y=[1036:1292]
W0 = RL                   # 262
ERC = W0 + M + 6          # 524 erase cols [524:780]
ADC = ERC + M             # 780 add cols [780:1036]
YC = ADC + M              # 1036


def _bcast_free(ap, reps, width):
    """View a [128, width] tile as [128, reps, width] with step-0 mid dim."""
    return bass.AP(tensor=ap.tensor, offset=ap.offset,
                   ap=[ap.ap[0], [0, reps], [1, width]])


def ntm_body(ctx, tc, io):
    nc = tc.nc
    sgl = ctx.enter_context(tc.tile_pool(name="sgl", bufs=1))
    mems = ctx.enter_context(tc.tile_pool(name="mems", bufs=3))
    us = ctx.enter_context(tc.tile_pool(name="us", bufs=2))
    small = ctx.enter_context(tc.tile_pool(name="small", bufs=2))
    scrp = ctx.enter_context(tc.tile_pool(name="scrp", bufs=2))
    addr = ctx.enter_context(tc.tile_pool(name="addr", bufs=1))
    ps_rv = ctx.enter_context(tc.tile_pool(name="ps_rv", bufs=1, space="PSUM"))
    ps_t = ctx.enter_context(tc.tile_pool(name="ps_t", bufs=2, space="PSUM"))
    ps_big = ctx.enter_context(tc.tile_pool(name="ps_big", bufs=2, space="PSUM"))
    ps_gru = ctx.enter_context(tc.tile_pool(name="ps_gru", bufs=1, space="PSUM"))

    ident = sgl.tile([128, 128], F32)
    masks.make_identity(nc, ident)
    ones_row = sgl.tile([1, 128], F32)
    nc.vector.memset(ones_row, 1.0)
    epsp = sgl.tile([32, 1], F32)
    nc.vector.memset(epsp, EPS)

    def pe_t(dst, src):
        """PE transpose src [P, F] (SBUF) -> psum [F, P], ACT copy -> dst."""
        p = src.shape[0]
        tp = ps_t.tile([src.free_size(), p], F32, tag="tp")
        nc.tensor.transpose(tp, src, ident[:p, :p])
        nc.scalar.copy(dst, tp)

    # ---- setup: weights (transposed via PE), biases, small inputs ----
    w_ihT = sgl.tile([128, 2, 3 * H], F32)
    w_hhT = sgl.tile([128, 2, 3 * H], F32)
    projT = sgl.tile([128, 2, OUTF], F32)
    for name, dst in (("w_ih", w_ihT), ("w_hh", w_hhT)):
        nat = scrp.tile([128, 6, M], F32, tag="wnat")
        nc.sync.dma_start(out=nat, in_=io[name].rearrange("(o p) m -> p o m", p=128))
        for o in range(6):
            for kc in range(2):
                pe_t(dst[:, kc, o * 128:(o + 1) * 128],
                     nat[:, o, kc * 128:(kc + 1) * 128])
    for half in range(2):
        pnat = scrp.tile([128, 5, H], F32, tag="wnat")
        r0 = half * 640
        nc.sync.dma_start(out=pnat,
                          in_=io["proj_w"][r0:r0 + 640, :].rearrange("(o p) m -> p o m", p=128))
        for o in range(5):
            oo = half * 5 + o
            for kc in range(2):
                pe_t(projT[:, kc, oo * 128:(oo + 1) * 128],
                     pnat[:, o, kc * 128:(kc + 1) * 128])
    pnat2 = sgl.tile([12, H], F32)
    nc.sync.dma_start(out=pnat2, in_=io["proj_w"][1280:OUTF, :])
    for kc in range(2):
        pe_t(projT[:, kc, 1280:OUTF], pnat2[:, kc * 128:(kc + 1) * 128])

    bih = sgl.tile([128, 6], F32)
    nc.sync.dma_start(out=bih, in_=io["b_ih"].rearrange("(o p) -> p o", p=128))
    bhh = sgl.tile([128, 6], F32)
    nc.sync.dma_start(out=bhh, in_=io["b_hh"].rearrange("(o p) -> p o", p=128))
    pb = sgl.tile([128, 11], F32)
    nc.sync.dma_start(out=pb[:, 0:10],
                      in_=io["proj_b"][0:1280].rearrange("(o p) -> p o", p=128))
    nc.sync.dma_start(out=pb[:12, 10:11], in_=io["proj_b"][1280:OUTF, None])

    x_sb = sgl.tile([BL, M], F32)
    nc.sync.dma_start(out=x_sb, in_=io["x"])
    h_sb = sgl.tile([BL, H], F32)
    nc.sync.dma_start(out=h_sb, in_=io["hidden"])
    rw_sb = sgl.tile([BL, N], F32)
    nc.sync.dma_start(out=rw_sb, in_=io["read_w"])
    ww_sb = sgl.tile([BL, N], F32)
    nc.sync.dma_start(out=ww_sb, in_=io["write_w"])

    rwT = sgl.tile([128, NC, BL], F32)
    for c in range(NC):
        pe_t(rwT[:, c, :], rw_sb[:, c * 128:(c + 1) * 128])
    xT = sgl.tile([128, 2, BL], F32)
    hT = sgl.tile([128, 2, BL], F32)
    for kc in range(2):
        pe_t(xT[:, kc, :], x_sb[:, kc * 128:(kc + 1) * 128])
        pe_t(hT[:, kc, :], h_sb[:, kc * 128:(kc + 1) * 128])

    # ---- phase A: read_vec + row norms ----
    rv_sb = sgl.tile([BL, M], F32)
    ncol = sgl.tile([128, NC, BL], F32)
    for b in range(BL):
        slab = mems.tile([128, NC, M], F32, tag="slab")
        nc.sync.dma_start(out=slab,
                          in_=io["memory"][b].rearrange("(c p) m -> p c m", p=128))
        rv_ps = ps_rv.tile([1, M], F32, tag="rv")
        for c in range(NC):
            nc.tensor.matmul(rv_ps, rwT[:, c, b:b + 1], slab[:, c, :],
                             start=(c == 0), stop=(c == NC - 1))
        rvstg = small.tile([1, M], F32, tag="rvstg")
        nc.scalar.copy(rvstg, rv_ps)
        nc.sync.dma_start(out=rv_sb[b:b + 1, :], in_=rvstg)
        for c in range(NC):
            scr_a = scrp.tile([128, M], F32, tag="scr_a")
            nc.scalar.activation(out=scr_a, in_=slab[:, c, :], func=AF.Square,
                                 accum_out=ncol[:, c, b:b + 1])

    # ---- GRU (feat-major layout [128, 2, BL]) ----
    rvT = sgl.tile([128, 2, BL], F32)
    for kc in range(2):
        pe_t(rvT[:, kc, :], rv_sb[:, kc * 128:(kc + 1) * 128])

    def gru_step(xTt, hTt, tag):
        gi_ps = ps_gru.tile([128, 6, BL], F32, tag="gi")
        gh_ps = ps_gru.tile([128, 6, BL], F32, tag="gh")
        for o in range(6):
            for kc in range(2):
                nc.tensor.matmul(gi_ps[:, o, :], w_ihT[:, kc, o * 128:(o + 1) * 128],
                                 xTt[:, kc, :], start=(kc == 0), stop=(kc == 1))
            for kc in range(2):
                nc.tensor.matmul(gh_ps[:, o, :], w_hhT[:, kc, o * 128:(o + 1) * 128],
                                 hTt[:, kc, :], start=(kc == 0), stop=(kc == 1))
        gisb = small.tile([128, 6, BL], F32, tag="gisb")
        for o in range(6):
            nc.scalar.activation(out=gisb[:, o, :], in_=gi_ps[:, o, :],
                                 func=AF.Identity, bias=bih[:, o:o + 1])
        rt = small.tile([128, 2, BL], F32, tag="rt")
        nc.vector.tensor_tensor(out=rt, in0=gisb[:, 0:2, :], in1=gh_ps[:, 0:2, :],
                                op=AL.add)
        r = small.tile([128, 2, BL], F32, tag="r")
        zt = small.tile([128, 2, BL], F32, tag="zt")
        nc.vector.tensor_tensor(out=zt, in0=gisb[:, 2:4, :], in1=gh_ps[:, 2:4, :],
                                op=AL.add)
        z = small.tile([128, 2, BL], F32, tag="z")
        hn = small.tile([128, 2, BL], F32, tag="hn")
        for j in range(2):
            nc.scalar.activation(out=r[:, j, :], in_=rt[:, j, :], func=AF.Sigmoid,
                                 bias=bhh[:, j:j + 1])
            nc.scalar.activation(out=z[:, j, :], in_=zt[:, j, :], func=AF.Sigmoid,
                                 bias=bhh[:, 2 + j:3 + j])
            nc.scalar.activation(out=hn[:, j, :], in_=gh_ps[:, 4 + j, :],
                                 func=AF.Identity, bias=bhh[:, 4 + j:5 + j])
        rn = small.tile([128, 2, BL], F32, tag="rn")
        nc.vector.tensor_tensor(out=rn, in0=r, in1=hn, op=AL.mult)
        tn = small.tile([128, 2, BL], F32, tag="tn")
        nc.vector.tensor_tensor(out=tn, in0=gisb[:, 4:6, :], in1=rn, op=AL.add)
        n_ = small.tile([128, 2, BL], F32, tag="n_")
        nc.scalar.activation(out=n_, in_=tn, func=AF.Tanh)
        dh = small.tile([128, 2, BL], F32, tag="dh")
        nc.vector.tensor_tensor(out=dh, in0=hTt, in1=n_, op=AL.subtract)
        zd = small.tile([128, 2, BL], F32, tag="zd")
        nc.vector.tensor_tensor(out=zd, in0=z, in1=dh, op=AL.mult)
        h2 = sgl.tile([128, 2, BL], F32, tag=f"h{tag}")
        nc.vector.tensor_tensor(out=h2, in0=n_, in1=zd, op=AL.add)
        return h2

    h2 = gru_step(xT, hT, "2")
    h3 = gru_step(rvT, h2, "3")

    # projection -> co_sb [BL, 1408]
    co_ps = ps_big.tile([128, 11, BL], F32, tag="big")
    for o in range(10):
        for kc in range(2):
            nc.tensor.matmul(co_ps[:, o, :], projT[:, kc, o * 128:(o + 1) * 128],
                             h3[:, kc, :], start=(kc == 0), stop=(kc == 1))
    for kc in range(2):
        nc.tensor.matmul(co_ps[:12, 10, :], projT[:, kc, 1280:OUTF],
                         h3[:, kc, :], start=(kc == 0), stop=(kc == 1))
    co_cols = sgl.tile([128, 11, BL], F32)
    for o in range(10):
        nc.scalar.activation(out=co_cols[:, o, :], in_=co_ps[:, o, :],
                             func=AF.Identity, bias=pb[:, o:o + 1])
    nc.scalar.activation(out=co_cols[:12, 10, :], in_=co_ps[:12, 10, :],
                         func=AF.Identity, bias=pb[:12, 10:11])
    co_sb = sgl.tile([BL, 11 * 128], F32)
    for o in range(10):
        pe_t(co_sb[:, o * 128:(o + 1) * 128], co_cols[:, o, :])
    pe_t(co_sb[:, 1280:OUTF], co_cols[:12, 10, :])
    nc.sync.dma_start(out=io["y"], in_=co_sb[:, YC:YC + M])

    # ---- head params ----
    ers = sgl.tile([BL, M], F32)
    nc.scalar.activation(out=ers, in_=co_sb[:, ERC:ERC + M], func=AF.Sigmoid)

    def head_sigmoid(c0):
        p0 = c0 + M
        g = sgl.tile([BL, 1], F32, tag=f"g{c0}")
        nc.scalar.activation(out=g, in_=co_sb[:, p0 + 1:p0 + 2], func=AF.Sigmoid)
        return g

    g_r = head_sigmoid(0)
    g_w = head_sigmoid(W0)

    def softplus_col(dst, col_ap):
        # ln(exp(x) + 1); inputs are O(1) so exp cannot overflow
        nc.scalar.activation(out=dst, in_=col_ap, func=AF.Exp)
        nc.scalar.activation(out=dst, in_=dst, func=AF.Ln, bias=1.0)

    def head_params(c0):
        key = co_sb[:, c0:c0 + M]
        scr_k = scrp.tile([BL, M], F32, tag="scr_k")
        k2 = small.tile([BL, 1], F32, tag="k2")
        nc.vector.scalar_tensor_tensor(out=scr_k, in0=key, scalar=1.0, in1=key,
                                       op0=AL.mult, op1=AL.mult, accum_out=k2)
        # 1/||key|| = exp(-0.5 ln(k2)); the reference's +eps is negligible
        rk = small.tile([BL, 1], F32, tag="rk")
        nc.scalar.activation(out=rk, in_=k2, func=AF.Ln)
        nc.scalar.activation(out=rk, in_=rk, func=AF.Exp, scale=-0.5)
        p0 = c0 + M
        beta = small.tile([BL, 1], F32, tag="beta")
        softplus_col(beta, co_sb[:, p0:p0 + 1])
        bscale = sgl.tile([BL, 1], F32, tag=f"bsc{c0}")
        nc.vector.tensor_tensor(out=bscale, in0=beta, in1=rk, op=AL.mult)
        gam = sgl.tile([BL, 1], F32, tag=f"gam{c0}")
        softplus_col(gam, co_sb[:, p0 + 5:p0 + 6])
        nc.vector.tensor_scalar_add(gam, gam, 1.0)
        mx3 = small.tile([BL, 1], F32, tag="mx3")
        nc.vector.reduce_max(out=mx3, in_=co_sb[:, p0 + 2:p0 + 5], axis=AX.X)
        nm3 = small.tile([BL, 1], F32, tag="nm3")
        nc.vector.tensor_scalar(out=nm3, in0=mx3, scalar1=-1.0, scalar2=None,
                                op0=AL.mult)
        e3 = small.tile([BL, 3], F32, tag="e3")
        nc.scalar.activation(out=e3, in_=co_sb[:, p0 + 2:p0 + 5], func=AF.Exp,
                             bias=nm3)
        s3s = small.tile([BL, 1], F32, tag="s3s")
        nc.vector.reduce_sum(out=s3s, in_=e3, axis=AX.X)
        rs3 = small.tile([BL, 1], F32, tag="rs3")
        nc.vector.reciprocal(rs3, s3s)
        sfx = sgl.tile([BL, 3], F32, tag=f"sfx{c0}")
        nc.vector.tensor_scalar(out=sfx, in0=e3, scalar1=rs3, scalar2=None,
                                op0=AL.mult)
        return bscale, gam, sfx

    bsc_r, gam_r, sfx_r = head_params(0)
    bsc_w, gam_w, sfx_w = head_params(W0)

    # ---- phase B1: cosine dots ----
    dcr = sgl.tile([128, NC, BL], F32)
    dcw = sgl.tile([128, NC, BL], F32)
    for b in range(BL):
        slab = mems.tile([128, NC, M], F32, tag="slab")
        nc.sync.dma_start(out=slab,
                          in_=io["memory"][b].rearrange("(c p) m -> p c m", p=128))
        kstg = small.tile([1, 2 * M], F32, tag="kstg")
        nc.sync.dma_start(out=kstg[:, 0:M], in_=co_sb[b:b + 1, 0:M])
        nc.sync.dma_start(out=kstg[:, M:2 * M], in_=co_sb[b:b + 1, W0:W0 + M])
        kb_ps = ps_big.tile([128, 2 * M], F32, tag="big")
        nc.tensor.matmul(kb_ps, ones_row, kstg, start=True, stop=True)
        for c in range(NC):
            scr1 = scrp.tile([128, M], F32, tag="scr1")
            nc.vector.scalar_tensor_tensor(
                out=scr1, in0=slab[:, c, :], scalar=1.0, in1=kb_ps[:, 0:M],
                op0=AL.mult, op1=AL.mult, accum_out=dcr[:, c, b:b + 1])
            scr2 = scrp.tile([128, M], F32, tag="scr2")
            nc.vector.scalar_tensor_tensor(
                out=scr2, in0=slab[:, c, :], scalar=1.0, in1=kb_ps[:, M:2 * M],
                op0=AL.mult, op1=AL.mult, accum_out=dcw[:, c, b:b + 1])

    # ---- cos scaling + transpose to [BL, N] ----
    rmc = sgl.tile([128, NC, BL], F32)
    nc.scalar.activation(out=rmc, in_=ncol, func=AF.Ln)
    nc.scalar.activation(out=rmc, in_=rmc, func=AF.Exp, scale=-0.5)
    cr = sgl.tile([128, NC, BL], F32)
    nc.vector.tensor_tensor(out=cr, in0=dcr, in1=rmc, op=AL.mult)
    cw = sgl.tile([128, NC, BL], F32)
    nc.vector.tensor_tensor(out=cw, in0=dcw, in1=rmc, op=AL.mult)

    dn_r = addr.tile([BL, N], F32, tag="dn_r")
    dn_w = addr.tile([BL, N], F32, tag="dn_w")
    for c in range(NC):
        pe_t(dn_r[:, c * 128:(c + 1) * 128], cr[:, c, :])
        pe_t(dn_w[:, c * 128:(c + 1) * 128], cw[:, c, :])

    # ---- addressing per head ----
    def address(dn, bscale, g, gam, sfx, prev, wtag):
        mx = small.tile([BL, 1], F32, tag="mx")
        nc.vector.reduce_max(out=mx, in_=dn, axis=AX.X)
        negb = small.tile([BL, 1], F32, tag="negb")
        nc.vector.tensor_scalar(out=negb, in0=mx, scalar1=bscale, scalar2=-1.0,
                                op0=AL.mult, op1=AL.mult)
        # e -> dn in place
        nc.scalar.activation(out=dn, in_=dn, func=AF.Exp, scale=bscale, bias=negb)
        ssum = small.tile([BL, 1], F32, tag="ssum")
        nc.vector.reduce_sum(out=ssum, in_=dn, axis=AX.X)
        rs = small.tile([BL, 1], F32, tag="rs")
        nc.vector.reciprocal(rs, ssum)
        t1 = addr.tile([BL, N], F32, tag="t1")
        nc.vector.tensor_scalar(out=t1, in0=dn, scalar1=rs, scalar2=None,
                                op0=AL.mult)                      # w1
        nc.vector.tensor_tensor(out=t1, in0=t1, in1=prev, op=AL.subtract)  # d1
        t2 = addr.tile([BL, N], F32, tag="t2")
        nc.vector.scalar_tensor_tensor(out=t2, in0=t1, scalar=g, in1=prev,
                                       op0=AL.mult, op1=AL.add)   # w2
        ext = addr.tile([BL, N + 2], F32, tag="ext")
        nc.vector.tensor_copy(out=ext[:, 1:N + 1], in_=t2)
        nc.vector.tensor_copy(out=ext[:, 0:1], in_=t2[:, N - 1:N])
        nc.vector.tensor_copy(out=ext[:, N + 1:N + 2], in_=t2[:, 0:1])
        nc.vector.tensor_scalar(out=t1, in0=ext[:, 2:N + 2], scalar1=sfx[:, 0:1],
                                scalar2=None, op0=AL.mult)        # a1
        nc.vector.scalar_tensor_tensor(out=t2, in0=ext[:, 1:N + 1],
                                       scalar=sfx[:, 1:2], in1=t1,
                                       op0=AL.mult, op1=AL.add)   # a2
        nc.vector.scalar_tensor_tensor(out=t1, in0=ext[:, 0:N],
                                       scalar=sfx[:, 2:3], in1=t2,
                                       op0=AL.mult, op1=AL.add)   # a3
        nc.scalar.activation(out=t2, in_=t1, func=AF.Ln, bias=epsp)   # lg
        nc.scalar.activation(out=t1, in_=t2, func=AF.Exp, scale=gam)  # pw
        psm = small.tile([BL, 1], F32, tag="psm")
        nc.vector.reduce_sum(out=psm, in_=t1, axis=AX.X)
        nc.vector.tensor_scalar_add(psm, psm, EPS)
        rp = small.tile([BL, 1], F32, tag="rp")
        nc.vector.reciprocal(rp, psm)
        wfin = addr.tile([BL, N], F32, tag=wtag)
        nc.vector.tensor_scalar(out=wfin, in0=t1, scalar1=rp, scalar2=None,
                                op0=AL.mult)
        return wfin

    wfin_r = address(dn_r, bsc_r, g_r, gam_r, sfx_r, rw_sb, "wf_r")
    nc.sync.dma_start(out=io["new_read_w"], in_=wfin_r)
    wfin_w = address(dn_w, bsc_w, g_w, gam_w, sfx_w, ww_sb, "wf_w")
    nc.sync.dma_start(out=io["new_write_w"], in_=wfin_w)

    wtcol = sgl.tile([128, NC, BL], F32)
    for c in range(NC):
        pe_t(wtcol[:, c, :], wfin_w[:, c * 128:(c + 1) * 128])

    # ---- phase B2: memory update ----
    for b in range(BL):
        slab = mems.tile([128, NC, M], F32, tag="slab")
        nc.sync.dma_start(out=slab,
                          in_=io["memory"][b].rearrange("(c p) m -> p c m", p=128))
        estg = small.tile([1, 2 * M], F32, tag="estg")
        nc.sync.dma_start(out=estg[:, 0:M], in_=ers[b:b + 1, :])
        nc.sync.dma_start(out=estg[:, M:2 * M], in_=co_sb[b:b + 1, ADC:ADC + M])
        ea_ps = ps_big.tile([128, 2 * M], F32, tag="big")
        nc.tensor.matmul(ea_ps, ones_row, estg, start=True, stop=True)
        erb = us.tile([128, M], F32, tag="erb")
        nc.scalar.copy(erb, ea_ps[:, 0:M])
        u = us.tile([128, NC, M], F32, tag="u")
        nc.gpsimd.tensor_tensor(out=u, in0=slab, in1=_bcast_free(erb, NC, M),
                                op=AL.mult)
        for c in range(NC):
            d = small.tile([128, M], F32, tag="d")
            nc.vector.tensor_tensor(out=d, in0=ea_ps[:, M:2 * M], in1=u[:, c, :],
                                    op=AL.subtract)
            nc.vector.scalar_tensor_tensor(out=slab[:, c, :], in0=d,
                                           scalar=wtcol[:, c, b:b + 1],
                                           in1=slab[:, c, :],
                                           op0=AL.mult, op1=AL.add)
        nc.sync.dma_start(out=io["new_memory"][b].rearrange("(c p) m -> p c m", p=128),
                          in_=slab)


@functools.lru_cache(maxsize=1)
def build_kernel():
    nc = bacc.Bacc("TRN2", target_bir_lowering=False, debug=False)
    io = {}
    for name, shp in [("x", [BL, M]), ("memory", [BL, N, M]), ("read_w", [BL, N]),
                      ("write_w", [BL, N]), ("hidden", [BL, H]),
                      ("w_ih", [3 * H, M]), ("w_hh", [3 * H, H]), ("b_ih", [3 * H]),
                      ("b_hh", [3 * H]), ("proj_w", [OUTF, H]), ("proj_b", [OUTF])]:
        io[name] = nc.dram_tensor(name, shp, F32, kind="ExternalInput").ap()
    for name, shp in [("y", [BL, M]), ("new_memory", [BL, N, M]),
                      ("new_read_w", [BL, N]), ("new_write_w", [BL, N])]:
        io[name] = nc.dram_tensor(name, shp, F32, kind="ExternalOutput").ap()
    with tile.TileContext(nc) as tc:
        with ExitStack() as ctx:
            ntm_body(ctx, tc, io)
    nc.compile()
    return nc


def shard_inputs(inputs):
    f = lambda a: np.ascontiguousarray(np.asarray(a), dtype=np.float32)
    x = f(inputs["x"]); memory = f(inputs["memory"])
    read_w = f(inputs["read_w"]).reshape(B, N)
    write_w = f(inputs["write_w"]).reshape(B, N)
    hidden = f(inputs["hidden"]).reshape(B, H)
    rep = {k: f(inputs[k]) for k in
           ("w_ih", "w_hh", "b_ih", "b_hh", "proj_w", "proj_b")}
    maps = []
    for c in range(NCORES):
        sl = slice(c * BL, (c + 1) * BL)
        m = {"x": x[sl], "memory": memory[sl], "read_w": read_w[sl],
             "write_w": write_w[sl], "hidden": hidden[sl]}
        m.update(rep)
        maps.append(m)
    return maps


def kernel(**inputs):
    from concourse.bass_utils import run_bass_kernel_spmd
    nc = build_kernel()
    in_maps = shard_inputs(inputs)
    res = run_bass_kernel_spmd(nc, in_maps, core_ids=list(range(NCORES)),
                               trace=bool(int(os.environ.get("NTM_TRACE", "0"))))
    outs = res.results
    y = np.concatenate([r["y"] for r in outs], axis=0)
    new_memory = np.concatenate([r["new_memory"] for r in outs], axis=0)
    new_read_w = np.concatenate([r["new_read_w"] for r in outs], axis=0)
    new_write_w = np.concatenate([r["new_write_w"] for r in outs], axis=0)
    return (y, new_memory, new_read_w.reshape(B, 1, N),
            new_write_w.reshape(B, 1, N))
